# revision 19
# baseline (speedup 1.0000x reference)
"""Trainium2 Bass kernel for nn_CascadingSystem (confidence-gated 2-expert blend).

Computation (reference):
    xf = x.reshape(256, 150528)
    t_out = xf @ W1 + b1            # [256, 2]
    f_out = xf @ W2 + b2            # [256, 2]
    conf  = max(softmax(t_out, 1), 1)
    out   = where(conf > 0.95, t_out, 0.7*t_out + 0.3*f_out)

Strategy (memory-bound; reading x dominates; ~358 GB/s HBM per core):
  - Shard the feature dim D=150528 across 8 cores (18816 each). Every core
    streams its d-slice of ALL 256 samples once from HBM and computes the
    partial logits on the tensor engine, 147 accumulating matmul chunks of
    K=128 per stream.
  - Precision/bandwidth: fp32 matmuls are too slow and 4 B/elem. Decompose
    on the host
        x = xh(fp16) + xr,   xr8 = fp8_e4m3(xr * 2^12)
        W = wh(fp16) + wl(fp16),  w8 = fp8_e4m3(W * 2^9)
        logits = xh*wh + xh*wl + (xr8*w8) / 2^21
    3 B/elem -> ~41.5us DMA floor. Max logit error ~1.2e-4 (elementwise
    rel err 1.96e-2 vs the 2e-2 gate -- budget fully spent, do not touch).
  - PE: 4-way column tiling (tile_position col groups 0/32/64/96). fp16
    chunk j accumulates in col group j%2 (psum partitions 0/32), fp8 chunk
    j in group 2+j%2 (partitions 64/96). Concurrent matmuls across groups
    take the PE off the critical path (stream rate 4 cols/cycle).
  - DMA: chunk groups sized [4,6,8,12,16*6,12,6,2,1]; each group is ONE
    DMA per HWDGE engine (fp16 piece on one, fp8 piece on the other,
    greedily byte-balanced). Cumulative per-engine semaphores; the PE
    waits per group on both engines' counters. Weights go first (w16 on
    sync, w8 on scalar).
  - Host sums the 8 cores' partial tensors and applies the tiny
    bias/softmax/threshold/blend epilogue on [256, 4] floats.
"""

from contextlib import ExitStack

import ml_dtypes
import numpy as np

import concourse.bass as bass
import concourse.mybir as mybir
from concourse.bass_utils import run_bass_kernel_spmd

NCORES = 8
B = 256            # batch (matmul moving dim)
D = 150528         # 3*224*224
DS = D // NCORES   # 18816 features per core
P = 128            # partitions / contraction tile
J = DS // P        # 147 matmul chunks per core
# j-chunks per DMA group; each group = one fp16 DMA + one fp8 DMA
GROUPS = [16, 16, 16, 16, 16, 16, 16, 16, 12, 4, 2, 1]
assert sum(GROUPS) == J
GSTARTS = [sum(GROUPS[:i]) for i in range(len(GROUPS))]
NG = len(GROUPS)
W16C = 8 * J       # fp16 weight cols (wh|wl, 4 each, per chunk)
W8C = 4 * J        # fp8 weight cols (4 per chunk)
T16 = W16C + J * B
T8 = W8C + J * B
XS = 2.0 ** 12     # fp8 residual scale
WS = 2.0 ** 9      # fp8 weight scale
THRESHOLD = 0.95

_CACHE = {}


def _build():
    nc = bass.Bass()
    x16_in = nc.declare_dram_parameter("x16", [P, T16], mybir.dt.float16, isOutput=False)
    x8_in = nc.declare_dram_parameter("x8", [P, T8], mybir.dt.float8e4, isOutput=False)
    out = nc.declare_dram_parameter(
        "partial", [P, B], mybir.dt.float32, isOutput=True
    )

    with ExitStack() as ctx:
        w16 = ctx.enter_context(nc.sbuf_tensor("w16", [P, W16C], mybir.dt.float16))
        w8 = ctx.enter_context(nc.sbuf_tensor("w8", [P, W8C], mybir.dt.float8e4))
        t16 = []
        t8 = []
        for g in range(NG):
            t16.append(
                ctx.enter_context(
                    nc.sbuf_tensor(f"t16_{g}", [P, GROUPS[g] * B], mybir.dt.float16)
                )
            )
            t8.append(
                ctx.enter_context(
                    nc.sbuf_tensor(f"t8_{g}", [P, GROUPS[g] * B], mybir.dt.float8e4)
                )
            )
        out_sb = ctx.enter_context(
            nc.sbuf_tensor("out_sb", [P, B], mybir.dt.float32)
        )
        # one FULL PSUM bank (512 fp32) per accumulation chain: start=True
        # clears has_written for the WHOLE bank, so chains must not share
        # a bank (a 1KB tensor could get co-allocated with another)
        acc16 = ctx.enter_context(nc.psum_tensor("acc16", [P, 512], mybir.dt.float32))
        acc8 = ctx.enter_context(nc.psum_tensor("acc8", [P, 512], mybir.dt.float32))
        acc_warm = ctx.enter_context(
            nc.psum_tensor("acc_warm", [P, 512], mybir.dt.float32)
        )
        warm_sb = ctx.enter_context(nc.sbuf_tensor("warm_sb", [P, 128], mybir.dt.float16))

        # Greedy byte-balance: per group, put the fp16 piece (2B/elem) on
        # the lighter engine, the fp8 piece on the other. Weights first:
        # w16 (301KB) on sync, w8 (75KB) on scalar.
        sync_bytes = 0
        scal_bytes = 0
        e16s = []
        for g in range(NG):
            b16 = GROUPS[g] * B * P * 2
            b8 = GROUPS[g] * B * P * 1
            if sync_bytes <= scal_bytes:
                e16s.append("sync")
                sync_bytes += b16
                scal_bytes += b8
            else:
                e16s.append("scal")
                scal_bytes += b16
                sync_bytes += b8

        # Per-DMA semaphores (a shared per-engine counter is UNSAFE: the 16
        # SDMA slots increment independently, so a cumulative value 16k does
        # not imply the k-th DMA fully landed). Allocate each engine's sems
        # contiguously so one range-clear covers them.
        s16g = [None] * NG
        s8g = [None] * NG
        sems_sync = []
        for g in range(NG):
            if e16s[g] == "sync":
                s16g[g] = ctx.enter_context(nc.semaphore(f"s16_{g}"))
                sems_sync.append(s16g[g])
            else:
                s8g[g] = ctx.enter_context(nc.semaphore(f"s8_{g}"))
                sems_sync.append(s8g[g])
        sw16 = ctx.enter_context(nc.semaphore("sw16"))
        sems_scal = []
        for g in range(NG):
            if e16s[g] == "sync":
                s8g[g] = ctx.enter_context(nc.semaphore(f"s8_{g}"))
                sems_scal.append(s8g[g])
            else:
                s16g[g] = ctx.enter_context(nc.semaphore(f"s16_{g}"))
                sems_scal.append(s16g[g])
        osem = ctx.enter_context(nc.semaphore("o"))
        sems_scal.append(osem)
        sw8 = ctx.enter_context(nc.semaphore("sw8"))
        pe_sem = ctx.enter_context(nc.semaphore("pe"))
        dve_sem = ctx.enter_context(nc.semaphore("dve"))

        def crange(sems):
            nums = sorted(s.num for s in sems)
            assert nums == list(range(nums[0], nums[-1] + 1))
            return range(nums[0], nums[-1] + 1)

        block = ctx.enter_context(nc.Block())

        # Self-initialization: semaphores persist across NEFF executions;
        # each engine clears the sems it increments, then all engines meet
        # at the NRT pseudo-barrier before any wait executes. The DMA
        # engines issue their first DMAs (weights + the first two groups)
        # BEFORE their barrier instruction: the increments land on sems the
        # same engine just cleared (program order), and every consumer waits
        # only after its own barrier -- so this is safe and starts the
        # stream ~1us earlier.
        nc.sync.sem_clear(crange(sems_sync + [sw16]))
        nc.scalar.sem_clear(crange(sems_scal + [sw8]))
        nc.tensor.sem_clear(pe_sem)
        nc.vector.sem_clear(dve_sem)

        def issue_x(eng, which, groups):
            for g in groups:
                c0 = GSTARTS[g] * B
                if e16s[g] == which:
                    eng.dma_start(
                        t16[g][:], x16_in[:, W16C + c0 : W16C + c0 + GROUPS[g] * B]
                    ).then_inc(s16g[g], 16)
                else:
                    eng.dma_start(
                        t8[g][:], x8_in[:, W8C + c0 : W8C + c0 + GROUPS[g] * B]
                    ).then_inc(s8g[g], 16)

        nc.sync.dma_start(w16[:], x16_in[:, 0:W16C]).then_inc(sw16, 16)
        nc.scalar.dma_start(w8[:], x8_in[:, 0:W8C]).then_inc(sw8, 16)
        issue_x(nc.sync, "sync", range(1))
        issue_x(nc.scalar, "scal", range(1))
        # PE warm-up burst pre-barrier (HAM un-throttle: 1.2 -> 2.4 GHz
        # before the real stream). warm_sb is never written -- junk values
        # stream into a scratch psum bank and are never read.
        for _ in range(26):
            nc.tensor.matmul(
                acc_warm[0:8, 0:128],
                warm_sb[:, 0:8],
                warm_sb[:, 0:128],
                start=True,
                stop=True,
                tile_position=(0, 0),
                skip_group_check=True,
            )
        nc._nrt_pseudo_barrier()

        @block.sync
        def _(sync):
            issue_x(sync, "sync", range(1, NG))

        @block.scalar
        def _(scalar):
            issue_x(scalar, "scal", range(1, NG))
            scalar.wait_ge(dve_sem, 1)
            scalar.dma_start(out[:], out_sb[:]).then_inc(osem, 16)
            scalar.wait_ge(osem, 16)

        @block.tensor
        def _(tensor):
            mm = None
            tensor.wait_ge(sw16, 16)
            tensor.wait_ge(sw8, 16)
            for g in range(NG):
                tensor.wait_ge(s16g[g], 16)
                tensor.wait_ge(s8g[g], 16)
                # strict per-chunk alternation: adjacent matmuls target
                # different col groups (0 / 64), so their streams overlap in
                # the array and the NX issue path is the only serial cost
                for jj in range(GROUPS[g]):
                    j = GSTARTS[g] + jj
                    tensor.matmul(
                        acc16[0:8, 0:B],
                        w16[:, 8 * j : 8 * j + 8],
                        t16[g][:, jj * B : jj * B + B],
                        start=(j == 0),
                        stop=(j == J - 1),
                        tile_position=(0, 0),
                        skip_group_check=True,
                    )
                    mm = tensor.matmul(
                        acc8[64:68, 0:B],
                        w8[:, 4 * j : 4 * j + 4],
                        t8[g][:, jj * B : jj * B + B],
                        start=(j == 0),
                        stop=(j == J - 1),
                        tile_position=(0, 64),
                        skip_group_check=True,
                    )
            mm.then_inc(pe_sem, 1)

        @block.vector
        def _(vector):
            # psum rows: 0:8 = fp16 partials (wh|wl), 64:68 = fp8 residual
            # partials (scaled by XS*WS)
            vector.wait_ge(pe_sem, 1)
            # small DVE-side delay: the matmul's sem update fires at retire,
            # ~230ns before the systolic drain lands in PSUM
            for _ in range(2):
                vector.tensor_copy(out_sb[0:1, 0:8], w16[0:1, 0:8])
            vector.tensor_copy(out_sb[0:8, :], acc16[0:8, 0:B])
            vector.tensor_copy(out_sb[64:68, :], acc8[64:68, 0:B]).then_inc(
                dve_sem, 1
            )

    return nc


def _pack(x, W1, W2):
    xf = np.ascontiguousarray(x, dtype=np.float32).reshape(B, D)
    xh = xf.astype(np.float16)
    xr8 = ((xf - xh.astype(np.float32)) * np.float32(XS)).astype(ml_dtypes.float8_e4m3)

    w4 = np.concatenate(
        [np.asarray(W1, np.float32), np.asarray(W2, np.float32)], axis=1
    )  # [D, 4]
    wh = w4.astype(np.float16)
    wl = (w4 - wh.astype(np.float32)).astype(np.float16)
    w8 = (w4 * np.float32(WS)).astype(ml_dtypes.float8_e4m3)

    xw16 = np.empty((NCORES, P, T16), dtype=np.float16)
    # fp16 W part: col 8j + h*4 + c = (wh,wl)[h][k*DS + j*P + p, c]
    wst = np.stack([wh, wl])  # [2, D, 4]
    xw16[:, :, :W16C] = (
        wst.reshape(2, NCORES, J, P, 4)
        .transpose(1, 3, 2, 0, 4)
        .reshape(NCORES, P, W16C)
    )
    # fp16 x part: col W16C + j*B + b = xh[b, k*DS + j*P + p]
    xw16[:, :, W16C:] = (
        xh.reshape(B, NCORES, J, P).transpose(1, 3, 2, 0).reshape(NCORES, P, J * B)
    )

    xw8 = np.empty((NCORES, P, T8), dtype=ml_dtypes.float8_e4m3)
    xw8[:, :, :W8C] = (
        w8.reshape(NCORES, J, P, 4).transpose(0, 2, 1, 3).reshape(NCORES, P, W8C)
    )
    xw8[:, :, W8C:] = (
        xr8.reshape(B, NCORES, J, P).transpose(1, 3, 2, 0).reshape(NCORES, P, J * B)
    )
    return xw16, xw8


def kernel(x, W1, b1, W2, b2, trace=False, trace_cores=None):
    if "nc" not in _CACHE:
        _CACHE["nc"] = _build()
    nc = _CACHE["nc"]

    xw16, xw8 = _pack(x, W1, W2)
    in_maps = [{"x16": xw16[k], "x8": xw8[k]} for k in range(NCORES)]
    kw = {"trace_cores": trace_cores} if trace_cores else {}
    res = run_bass_kernel_spmd(nc, in_maps, list(range(NCORES)), trace=trace, **kw)
    _CACHE["last_results"] = res

    logits4 = np.zeros((4, B), dtype=np.float64)
    for k in range(NCORES):
        r = res.results[k]["partial"]  # [128, 256]
        logits4 += r[0:4] + r[4:8]
        logits4 += r[64:68].astype(np.float64) / (XS * WS)
    logits4 = logits4.astype(np.float32)

    t_out = logits4[0:2].T + np.asarray(b1, np.float32)  # [256, 2]
    f_out = logits4[2:4].T + np.asarray(b2, np.float32)  # [256, 2]
    m = t_out.max(axis=1, keepdims=True)
    e = np.exp(t_out - m)
    conf = (e / e.sum(axis=1, keepdims=True)).max(axis=1)
    blended = 0.7 * t_out + 0.3 * f_out
    out = np.where((conf > THRESHOLD)[:, None], t_out, blended)
    return out.astype(np.float32)


# revision 20
# speedup vs baseline: 1.0273x; 1.0273x over previous
"""Trainium2 Bass kernel for nn_CascadingSystem (confidence-gated 2-expert blend).

Computation (reference):
    xf = x.reshape(256, 150528)
    t_out = xf @ W1 + b1            # [256, 2]
    f_out = xf @ W2 + b2            # [256, 2]
    conf  = max(softmax(t_out, 1), 1)
    out   = where(conf > 0.95, t_out, 0.7*t_out + 0.3*f_out)

Strategy (memory-bound; reading x dominates; ~358 GB/s HBM per core):
  - Shard the feature dim D=150528 across 8 cores (18816 each). Every core
    streams its d-slice of ALL 256 samples once from HBM and computes the
    partial logits on the tensor engine, 147 accumulating matmul chunks of
    K=128 per stream.
  - Precision/bandwidth: fp32 matmuls are too slow and 4 B/elem. Decompose
    on the host
        x = xh(fp16) + xr,   xr8 = fp8_e4m3(xr * 2^12)
        W = wh(fp16) + wl(fp16),  w8 = fp8_e4m3(W * 2^9)
        logits = xh*wh + xh*wl + (xr8*w8) / 2^21
    3 B/elem -> ~41.5us DMA floor. Max logit error ~1.2e-4 (elementwise
    rel err 1.96e-2 vs the 2e-2 gate -- budget fully spent, do not touch).
  - PE: 4-way column tiling (tile_position col groups 0/32/64/96). fp16
    chunk j accumulates in col group j%2 (psum partitions 0/32), fp8 chunk
    j in group 2+j%2 (partitions 64/96). Concurrent matmuls across groups
    take the PE off the critical path (stream rate 4 cols/cycle).
  - DMA: chunk groups sized [4,6,8,12,16*6,12,6,2,1]; each group is ONE
    DMA per HWDGE engine (fp16 piece on one, fp8 piece on the other,
    greedily byte-balanced). Cumulative per-engine semaphores; the PE
    waits per group on both engines' counters. Weights go first (w16 on
    sync, w8 on scalar).
  - Host sums the 8 cores' partial tensors and applies the tiny
    bias/softmax/threshold/blend epilogue on [256, 4] floats.
"""

from contextlib import ExitStack

import ml_dtypes
import numpy as np

import concourse.bass as bass
import concourse.mybir as mybir
from concourse.bass_utils import run_bass_kernel_spmd

NCORES = 8
B = 256            # batch (matmul moving dim)
D = 150528         # 3*224*224
DS = D // NCORES   # 18816 features per core
P = 128            # partitions / contraction tile
J = DS // P        # 147 matmul chunks per core
# j-chunks per DMA group; each group = one fp16 DMA + one fp8 DMA
GROUPS = [16, 16, 16, 16, 16, 16, 16, 16, 12, 4, 2, 1]
assert sum(GROUPS) == J
GSTARTS = [sum(GROUPS[:i]) for i in range(len(GROUPS))]
NG = len(GROUPS)
W16C = 8 * J       # fp16 weight cols (wh|wl, 4 each, per chunk)
W8C = 4 * J        # fp8 weight cols (4 per chunk)
T16 = W16C + J * B
T8 = W8C + J * B
XS = 2.0 ** 12     # fp8 residual scale
WS = 2.0 ** 9      # fp8 weight scale
THRESHOLD = 0.95

_CACHE = {}


def _build():
    nc = bass.Bass()
    x16_in = nc.declare_dram_parameter("x16", [P, T16], mybir.dt.float16, isOutput=False)
    x8_in = nc.declare_dram_parameter("x8", [P, T8], mybir.dt.float8e4, isOutput=False)
    out = nc.declare_dram_parameter(
        "partial", [P, B], mybir.dt.float32, isOutput=True
    )

    with ExitStack() as ctx:
        w16 = ctx.enter_context(nc.sbuf_tensor("w16", [P, W16C], mybir.dt.float16))
        w8 = ctx.enter_context(nc.sbuf_tensor("w8", [P, W8C], mybir.dt.float8e4))
        t16 = []
        t8 = []
        for g in range(NG):
            t16.append(
                ctx.enter_context(
                    nc.sbuf_tensor(f"t16_{g}", [P, GROUPS[g] * B], mybir.dt.float16)
                )
            )
            t8.append(
                ctx.enter_context(
                    nc.sbuf_tensor(f"t8_{g}", [P, GROUPS[g] * B], mybir.dt.float8e4)
                )
            )
        out_sb = ctx.enter_context(
            nc.sbuf_tensor("out_sb", [P, B], mybir.dt.float32)
        )
        # one FULL PSUM bank (512 fp32) per accumulation chain: start=True
        # clears has_written for the WHOLE bank, so chains must not share
        # a bank (a 1KB tensor could get co-allocated with another)
        acc16 = ctx.enter_context(nc.psum_tensor("acc16", [P, 512], mybir.dt.float32))
        acc8 = ctx.enter_context(nc.psum_tensor("acc8", [P, 512], mybir.dt.float32))
        acc_warm = ctx.enter_context(
            nc.psum_tensor("acc_warm", [P, 512], mybir.dt.float32)
        )
        warm_sb = ctx.enter_context(nc.sbuf_tensor("warm_sb", [P, 128], mybir.dt.float16))

        # Greedy byte-balance: per group, put the fp16 piece (2B/elem) on
        # the lighter engine, the fp8 piece on the other. Weights first:
        # w16 (301KB) on sync, w8 (75KB) on scalar.
        sync_bytes = 0
        scal_bytes = 0
        e16s = []
        for g in range(NG):
            b16 = GROUPS[g] * B * P * 2
            b8 = GROUPS[g] * B * P * 1
            if sync_bytes <= scal_bytes:
                e16s.append("sync")
                sync_bytes += b16
                scal_bytes += b8
            else:
                e16s.append("scal")
                scal_bytes += b16
                sync_bytes += b8

        # Per-DMA semaphores (a shared per-engine counter is UNSAFE: the 16
        # SDMA slots increment independently, so a cumulative value 16k does
        # not imply the k-th DMA fully landed). Allocate each engine's sems
        # contiguously so one range-clear covers them.
        s16g = [None] * NG
        s8g = [None] * NG
        sems_sync = []
        for g in range(NG):
            if e16s[g] == "sync":
                s16g[g] = ctx.enter_context(nc.semaphore(f"s16_{g}"))
                sems_sync.append(s16g[g])
            else:
                s8g[g] = ctx.enter_context(nc.semaphore(f"s8_{g}"))
                sems_sync.append(s8g[g])
        sw16 = ctx.enter_context(nc.semaphore("sw16"))
        sems_scal = []
        for g in range(NG):
            if e16s[g] == "sync":
                s8g[g] = ctx.enter_context(nc.semaphore(f"s8_{g}"))
                sems_scal.append(s8g[g])
            else:
                s16g[g] = ctx.enter_context(nc.semaphore(f"s16_{g}"))
                sems_scal.append(s16g[g])
        osem = ctx.enter_context(nc.semaphore("o"))
        sems_scal.append(osem)
        sw8 = ctx.enter_context(nc.semaphore("sw8"))
        pe_sem = ctx.enter_context(nc.semaphore("pe"))
        dve_sem = ctx.enter_context(nc.semaphore("dve"))

        def crange(sems):
            nums = sorted(s.num for s in sems)
            assert nums == list(range(nums[0], nums[-1] + 1))
            return range(nums[0], nums[-1] + 1)

        block = ctx.enter_context(nc.Block())

        # Self-initialization: semaphores persist across NEFF executions;
        # each engine clears the sems it increments, then all engines meet
        # at the NRT pseudo-barrier before any wait executes. The DMA
        # engines issue their first DMAs (weights + the first two groups)
        # BEFORE their barrier instruction: the increments land on sems the
        # same engine just cleared (program order), and every consumer waits
        # only after its own barrier -- so this is safe and starts the
        # stream ~1us earlier.
        nc.sync.sem_clear(crange(sems_sync + [sw16]))
        nc.scalar.sem_clear(crange(sems_scal + [sw8]))
        nc.tensor.sem_clear(pe_sem)
        nc.vector.sem_clear(dve_sem)

        def issue_x(eng, which, groups):
            for g in groups:
                c0 = GSTARTS[g] * B
                if e16s[g] == which:
                    eng.dma_start(
                        t16[g][:], x16_in[:, W16C + c0 : W16C + c0 + GROUPS[g] * B]
                    ).then_inc(s16g[g], 16)
                else:
                    eng.dma_start(
                        t8[g][:], x8_in[:, W8C + c0 : W8C + c0 + GROUPS[g] * B]
                    ).then_inc(s8g[g], 16)

        nc.sync.dma_start(w16[:], x16_in[:, 0:W16C]).then_inc(sw16, 16)
        nc.scalar.dma_start(w8[:], x8_in[:, 0:W8C]).then_inc(sw8, 16)
        issue_x(nc.sync, "sync", range(1))
        issue_x(nc.scalar, "scal", range(1))
        nc._nrt_pseudo_barrier()

        @block.sync
        def _(sync):
            issue_x(sync, "sync", range(1, NG))

        @block.scalar
        def _(scalar):
            issue_x(scalar, "scal", range(1, NG))
            scalar.wait_ge(dve_sem, 1)
            scalar.dma_start(out[:], out_sb[:]).then_inc(osem, 16)
            scalar.wait_ge(osem, 16)

        @block.tensor
        def _(tensor):
            # short post-barrier warm-up (HAM un-throttle toward 2.4 GHz)
            # sized to finish about when the weights + group-0 data land;
            # warm_sb is never written -- junk values stream into a scratch
            # psum bank and are never read
            for _ in range(10):
                tensor.matmul(
                    acc_warm[0:8, 0:128],
                    warm_sb[:, 0:8],
                    warm_sb[:, 0:128],
                    start=True,
                    stop=True,
                    tile_position=(0, 0),
                    skip_group_check=True,
                )
            mm = None
            tensor.wait_ge(sw16, 16)
            tensor.wait_ge(sw8, 16)
            for g in range(NG):
                tensor.wait_ge(s16g[g], 16)
                tensor.wait_ge(s8g[g], 16)
                # strict per-chunk alternation: adjacent matmuls target
                # different col groups (0 / 64), so their streams overlap in
                # the array and the NX issue path is the only serial cost
                for jj in range(GROUPS[g]):
                    j = GSTARTS[g] + jj
                    tensor.matmul(
                        acc16[0:8, 0:B],
                        w16[:, 8 * j : 8 * j + 8],
                        t16[g][:, jj * B : jj * B + B],
                        start=(j == 0),
                        stop=(j == J - 1),
                        tile_position=(0, 0),
                        skip_group_check=True,
                    )
                    mm = tensor.matmul(
                        acc8[64:68, 0:B],
                        w8[:, 4 * j : 4 * j + 4],
                        t8[g][:, jj * B : jj * B + B],
                        start=(j == 0),
                        stop=(j == J - 1),
                        tile_position=(0, 64),
                        skip_group_check=True,
                    )
            mm.then_inc(pe_sem, 1)

        @block.vector
        def _(vector):
            # psum rows: 0:8 = fp16 partials (wh|wl), 64:68 = fp8 residual
            # partials (scaled by XS*WS)
            vector.wait_ge(pe_sem, 1)
            # small DVE-side delay: the matmul's sem update fires at retire,
            # ~230ns before the systolic drain lands in PSUM
            for _ in range(2):
                vector.tensor_copy(out_sb[0:1, 0:8], w16[0:1, 0:8])
            vector.tensor_copy(out_sb[0:8, :], acc16[0:8, 0:B])
            vector.tensor_copy(out_sb[64:68, :], acc8[64:68, 0:B]).then_inc(
                dve_sem, 1
            )

    return nc


def _pack(x, W1, W2):
    xf = np.ascontiguousarray(x, dtype=np.float32).reshape(B, D)
    xh = xf.astype(np.float16)
    xr8 = ((xf - xh.astype(np.float32)) * np.float32(XS)).astype(ml_dtypes.float8_e4m3)

    w4 = np.concatenate(
        [np.asarray(W1, np.float32), np.asarray(W2, np.float32)], axis=1
    )  # [D, 4]
    wh = w4.astype(np.float16)
    wl = (w4 - wh.astype(np.float32)).astype(np.float16)
    w8 = (w4 * np.float32(WS)).astype(ml_dtypes.float8_e4m3)

    xw16 = np.empty((NCORES, P, T16), dtype=np.float16)
    # fp16 W part: col 8j + h*4 + c = (wh,wl)[h][k*DS + j*P + p, c]
    wst = np.stack([wh, wl])  # [2, D, 4]
    xw16[:, :, :W16C] = (
        wst.reshape(2, NCORES, J, P, 4)
        .transpose(1, 3, 2, 0, 4)
        .reshape(NCORES, P, W16C)
    )
    # fp16 x part: col W16C + j*B + b = xh[b, k*DS + j*P + p]
    xw16[:, :, W16C:] = (
        xh.reshape(B, NCORES, J, P).transpose(1, 3, 2, 0).reshape(NCORES, P, J * B)
    )

    xw8 = np.empty((NCORES, P, T8), dtype=ml_dtypes.float8_e4m3)
    xw8[:, :, :W8C] = (
        w8.reshape(NCORES, J, P, 4).transpose(0, 2, 1, 3).reshape(NCORES, P, W8C)
    )
    xw8[:, :, W8C:] = (
        xr8.reshape(B, NCORES, J, P).transpose(1, 3, 2, 0).reshape(NCORES, P, J * B)
    )
    return xw16, xw8


def kernel(x, W1, b1, W2, b2, trace=False, trace_cores=None):
    if "nc" not in _CACHE:
        _CACHE["nc"] = _build()
    nc = _CACHE["nc"]

    xw16, xw8 = _pack(x, W1, W2)
    in_maps = [{"x16": xw16[k], "x8": xw8[k]} for k in range(NCORES)]
    kw = {"trace_cores": trace_cores} if trace_cores else {}
    res = run_bass_kernel_spmd(nc, in_maps, list(range(NCORES)), trace=trace, **kw)
    _CACHE["last_results"] = res

    logits4 = np.zeros((4, B), dtype=np.float64)
    for k in range(NCORES):
        r = res.results[k]["partial"]  # [128, 256]
        logits4 += r[0:4] + r[4:8]
        logits4 += r[64:68].astype(np.float64) / (XS * WS)
    logits4 = logits4.astype(np.float32)

    t_out = logits4[0:2].T + np.asarray(b1, np.float32)  # [256, 2]
    f_out = logits4[2:4].T + np.asarray(b2, np.float32)  # [256, 2]
    m = t_out.max(axis=1, keepdims=True)
    e = np.exp(t_out - m)
    conf = (e / e.sum(axis=1, keepdims=True)).max(axis=1)
    blended = 0.7 * t_out + 0.3 * f_out
    out = np.where((conf > THRESHOLD)[:, None], t_out, blended)
    return out.astype(np.float32)


# revision 21
# speedup vs baseline: 1.1144x; 1.0848x over previous
"""Trainium2 Bass kernel for nn_CascadingSystem (confidence-gated 2-expert blend).

Computation (reference):
    xf = x.reshape(256, 150528)
    t_out = xf @ W1 + b1            # [256, 2]
    f_out = xf @ W2 + b2            # [256, 2]
    conf  = max(softmax(t_out, 1), 1)
    out   = where(conf > 0.95, t_out, 0.7*t_out + 0.3*f_out)

Strategy (memory-bound; reading x dominates; ~358 GB/s HBM per core):
  - Shard the feature dim D=150528 across 8 cores (18816 each). Every core
    streams its d-slice of ALL 256 samples once from HBM and computes the
    partial logits on the tensor engine, 147 accumulating matmul chunks of
    K=128 per stream.
  - Precision/bandwidth: fp32 matmuls are too slow and 4 B/elem. Decompose
    on the host
        x = xh(fp16) + xr,   xr8 = fp8_e4m3(xr * 2^12)
        W = wh(fp16) + wl(fp16),  w8 = fp8_e4m3(W * 2^9)
        logits = xh*wh + xh*wl + (xr8*w8) / 2^21
    3 B/elem -> ~41.5us DMA floor. Max logit error ~1.2e-4 (elementwise
    rel err 1.96e-2 vs the 2e-2 gate -- budget fully spent, do not touch).
  - PE: 4-way column tiling (tile_position col groups 0/32/64/96). fp16
    chunk j accumulates in col group j%2 (psum partitions 0/32), fp8 chunk
    j in group 2+j%2 (partitions 64/96). Concurrent matmuls across groups
    take the PE off the critical path (stream rate 4 cols/cycle).
  - DMA: chunk groups sized [4,6,8,12,16*6,12,6,2,1]; each group is ONE
    DMA per HWDGE engine (fp16 piece on one, fp8 piece on the other,
    greedily byte-balanced). Cumulative per-engine semaphores; the PE
    waits per group on both engines' counters. Weights go first (w16 on
    sync, w8 on scalar).
  - Host sums the 8 cores' partial tensors and applies the tiny
    bias/softmax/threshold/blend epilogue on [256, 4] floats.
"""

from contextlib import ExitStack

import ml_dtypes
import numpy as np

import concourse.bass as bass
import concourse.mybir as mybir
from concourse.bass_utils import run_bass_kernel_spmd

NCORES = 8
B = 256            # batch (matmul moving dim)
D = 150528         # 3*224*224
DS = D // NCORES   # 18816 features per core
P = 128            # partitions / contraction tile
J = DS // P        # 147 matmul chunks per core
# j-chunks per DMA group; each group = one fp16 DMA + one fp8 DMA
GROUPS = [16, 16, 16, 16, 16, 16, 16, 16, 13, 6]
assert sum(GROUPS) == J
GSTARTS = [sum(GROUPS[:i]) for i in range(len(GROUPS))]
NG = len(GROUPS)
W16C = 8 * J       # fp16 weight cols (wh|wl, 4 each, per chunk)
W8C = 4 * J        # fp8 weight cols (4 per chunk)
T16 = W16C + J * B
T8 = W8C + J * B
XS = 2.0 ** 12     # fp8 residual scale
WS = 2.0 ** 9      # fp8 weight scale
THRESHOLD = 0.95

_CACHE = {}


def _build():
    nc = bass.Bass()
    x16_in = nc.declare_dram_parameter("x16", [P, T16], mybir.dt.float16, isOutput=False)
    x8_in = nc.declare_dram_parameter("x8", [P, T8], mybir.dt.float8e4, isOutput=False)
    out = nc.declare_dram_parameter(
        "partial", [P, B], mybir.dt.float32, isOutput=True
    )

    with ExitStack() as ctx:
        w16 = ctx.enter_context(nc.sbuf_tensor("w16", [P, W16C], mybir.dt.float16))
        w8 = ctx.enter_context(nc.sbuf_tensor("w8", [P, W8C], mybir.dt.float8e4))
        t16 = []
        t8 = []
        for g in range(NG):
            t16.append(
                ctx.enter_context(
                    nc.sbuf_tensor(f"t16_{g}", [P, GROUPS[g] * B], mybir.dt.float16)
                )
            )
            t8.append(
                ctx.enter_context(
                    nc.sbuf_tensor(f"t8_{g}", [P, GROUPS[g] * B], mybir.dt.float8e4)
                )
            )
        out_sb = ctx.enter_context(
            nc.sbuf_tensor("out_sb", [P, B], mybir.dt.float32)
        )
        # one FULL PSUM bank (512 fp32) per accumulation chain: start=True
        # clears has_written for the WHOLE bank, so chains must not share
        # a bank (a 1KB tensor could get co-allocated with another)
        acc16 = ctx.enter_context(nc.psum_tensor("acc16", [P, 512], mybir.dt.float32))
        acc8 = ctx.enter_context(nc.psum_tensor("acc8", [P, 512], mybir.dt.float32))
        acc_warm = ctx.enter_context(
            nc.psum_tensor("acc_warm", [P, 512], mybir.dt.float32)
        )
        warm_sb = ctx.enter_context(nc.sbuf_tensor("warm_sb", [P, 128], mybir.dt.float16))

        # Greedy byte-balance: per group, put the fp16 piece (2B/elem) on
        # the lighter engine, the fp8 piece on the other. Weights first:
        # w16 (301KB) on sync, w8 (75KB) on scalar.
        sync_bytes = 0
        scal_bytes = 0
        e16s = []
        for g in range(NG):
            b16 = GROUPS[g] * B * P * 2
            b8 = GROUPS[g] * B * P * 1
            if sync_bytes <= scal_bytes:
                e16s.append("sync")
                sync_bytes += b16
                scal_bytes += b8
            else:
                e16s.append("scal")
                scal_bytes += b16
                sync_bytes += b8

        # Per-DMA semaphores (a shared per-engine counter is UNSAFE: the 16
        # SDMA slots increment independently, so a cumulative value 16k does
        # not imply the k-th DMA fully landed). Allocate each engine's sems
        # contiguously so one range-clear covers them.
        s16g = [None] * NG
        s8g = [None] * NG
        sems_sync = []
        for g in range(NG):
            if e16s[g] == "sync":
                s16g[g] = ctx.enter_context(nc.semaphore(f"s16_{g}"))
                sems_sync.append(s16g[g])
            else:
                s8g[g] = ctx.enter_context(nc.semaphore(f"s8_{g}"))
                sems_sync.append(s8g[g])
        sw16 = ctx.enter_context(nc.semaphore("sw16"))
        sems_scal = []
        for g in range(NG):
            if e16s[g] == "sync":
                s8g[g] = ctx.enter_context(nc.semaphore(f"s8_{g}"))
                sems_scal.append(s8g[g])
            else:
                s16g[g] = ctx.enter_context(nc.semaphore(f"s16_{g}"))
                sems_scal.append(s16g[g])
        osem = ctx.enter_context(nc.semaphore("o"))
        sems_scal.append(osem)
        sw8 = ctx.enter_context(nc.semaphore("sw8"))
        pe_sem = ctx.enter_context(nc.semaphore("pe"))
        dve_sem = ctx.enter_context(nc.semaphore("dve"))

        def crange(sems):
            nums = sorted(s.num for s in sems)
            assert nums == list(range(nums[0], nums[-1] + 1))
            return range(nums[0], nums[-1] + 1)

        block = ctx.enter_context(nc.Block())

        # Self-initialization: semaphores persist across NEFF executions;
        # each engine clears the sems it increments, then all engines meet
        # at the NRT pseudo-barrier before any wait executes. The DMA
        # engines issue their first DMAs (weights + the first two groups)
        # BEFORE their barrier instruction: the increments land on sems the
        # same engine just cleared (program order), and every consumer waits
        # only after its own barrier -- so this is safe and starts the
        # stream ~1us earlier.
        nc.sync.sem_clear(crange(sems_sync + [sw16]))
        nc.scalar.sem_clear(crange(sems_scal + [sw8]))
        nc.tensor.sem_clear(pe_sem)
        nc.vector.sem_clear(dve_sem)

        def issue_x(eng, which, groups):
            for g in groups:
                c0 = GSTARTS[g] * B
                if e16s[g] == which:
                    eng.dma_start(
                        t16[g][:], x16_in[:, W16C + c0 : W16C + c0 + GROUPS[g] * B]
                    ).then_inc(s16g[g], 16)
                else:
                    eng.dma_start(
                        t8[g][:], x8_in[:, W8C + c0 : W8C + c0 + GROUPS[g] * B]
                    ).then_inc(s8g[g], 16)

        nc.sync.dma_start(w16[:], x16_in[:, 0:W16C]).then_inc(sw16, 16)
        nc.scalar.dma_start(w8[:], x8_in[:, 0:W8C]).then_inc(sw8, 16)
        issue_x(nc.sync, "sync", range(1))
        issue_x(nc.scalar, "scal", range(1))
        nc._nrt_pseudo_barrier()

        @block.sync
        def _(sync):
            issue_x(sync, "sync", range(1, NG))

        @block.scalar
        def _(scalar):
            issue_x(scalar, "scal", range(1, NG))
            scalar.wait_ge(dve_sem, 1)
            scalar.dma_start(out[:], out_sb[:]).then_inc(osem, 16)
            scalar.wait_ge(osem, 16)

        @block.tensor
        def _(tensor):
            # short post-barrier warm-up (HAM un-throttle toward 2.4 GHz)
            # sized to finish about when the weights + group-0 data land;
            # warm_sb is never written -- junk values stream into a scratch
            # psum bank and are never read
            for _ in range(10):
                tensor.matmul(
                    acc_warm[0:8, 0:128],
                    warm_sb[:, 0:8],
                    warm_sb[:, 0:128],
                    start=True,
                    stop=True,
                    tile_position=(0, 0),
                    skip_group_check=True,
                )
            mm = None
            tensor.wait_ge(sw16, 16)
            tensor.wait_ge(sw8, 16)
            for g in range(NG):
                tensor.wait_ge(s16g[g], 16)
                tensor.wait_ge(s8g[g], 16)
                # strict per-chunk alternation: adjacent matmuls target
                # different col groups (0 / 64), so their streams overlap in
                # the array and the NX issue path is the only serial cost
                for jj in range(GROUPS[g]):
                    j = GSTARTS[g] + jj
                    tensor.matmul(
                        acc16[0:8, 0:B],
                        w16[:, 8 * j : 8 * j + 8],
                        t16[g][:, jj * B : jj * B + B],
                        start=(j == 0),
                        stop=(j == J - 1),
                        tile_position=(0, 0),
                        skip_group_check=True,
                    )
                    mm = tensor.matmul(
                        acc8[64:68, 0:B],
                        w8[:, 4 * j : 4 * j + 4],
                        t8[g][:, jj * B : jj * B + B],
                        start=(j == 0),
                        stop=(j == J - 1),
                        tile_position=(0, 64),
                        skip_group_check=True,
                    )
            mm.then_inc(pe_sem, 1)

        @block.vector
        def _(vector):
            # psum rows: 0:8 = fp16 partials (wh|wl), 64:68 = fp8 residual
            # partials (scaled by XS*WS)
            vector.wait_ge(pe_sem, 1)
            # small DVE-side delay: the matmul's sem update fires at retire,
            # ~230ns before the systolic drain lands in PSUM
            for _ in range(2):
                vector.tensor_copy(out_sb[0:1, 0:8], w16[0:1, 0:8])
            vector.tensor_copy(out_sb[0:8, :], acc16[0:8, 0:B])
            vector.tensor_copy(out_sb[64:68, :], acc8[64:68, 0:B]).then_inc(
                dve_sem, 1
            )

    return nc


def _pack(x, W1, W2):
    xf = np.ascontiguousarray(x, dtype=np.float32).reshape(B, D)
    xh = xf.astype(np.float16)
    xr8 = ((xf - xh.astype(np.float32)) * np.float32(XS)).astype(ml_dtypes.float8_e4m3)

    w4 = np.concatenate(
        [np.asarray(W1, np.float32), np.asarray(W2, np.float32)], axis=1
    )  # [D, 4]
    wh = w4.astype(np.float16)
    wl = (w4 - wh.astype(np.float32)).astype(np.float16)
    w8 = (w4 * np.float32(WS)).astype(ml_dtypes.float8_e4m3)

    xw16 = np.empty((NCORES, P, T16), dtype=np.float16)
    # fp16 W part: col 8j + h*4 + c = (wh,wl)[h][k*DS + j*P + p, c]
    wst = np.stack([wh, wl])  # [2, D, 4]
    xw16[:, :, :W16C] = (
        wst.reshape(2, NCORES, J, P, 4)
        .transpose(1, 3, 2, 0, 4)
        .reshape(NCORES, P, W16C)
    )
    # fp16 x part: col W16C + j*B + b = xh[b, k*DS + j*P + p]
    xw16[:, :, W16C:] = (
        xh.reshape(B, NCORES, J, P).transpose(1, 3, 2, 0).reshape(NCORES, P, J * B)
    )

    xw8 = np.empty((NCORES, P, T8), dtype=ml_dtypes.float8_e4m3)
    xw8[:, :, :W8C] = (
        w8.reshape(NCORES, J, P, 4).transpose(0, 2, 1, 3).reshape(NCORES, P, W8C)
    )
    xw8[:, :, W8C:] = (
        xr8.reshape(B, NCORES, J, P).transpose(1, 3, 2, 0).reshape(NCORES, P, J * B)
    )
    return xw16, xw8


def kernel(x, W1, b1, W2, b2, trace=False, trace_cores=None):
    if "nc" not in _CACHE:
        _CACHE["nc"] = _build()
    nc = _CACHE["nc"]

    xw16, xw8 = _pack(x, W1, W2)
    in_maps = [{"x16": xw16[k], "x8": xw8[k]} for k in range(NCORES)]
    kw = {"trace_cores": trace_cores} if trace_cores else {}
    res = run_bass_kernel_spmd(nc, in_maps, list(range(NCORES)), trace=trace, **kw)
    _CACHE["last_results"] = res

    logits4 = np.zeros((4, B), dtype=np.float64)
    for k in range(NCORES):
        r = res.results[k]["partial"]  # [128, 256]
        logits4 += r[0:4] + r[4:8]
        logits4 += r[64:68].astype(np.float64) / (XS * WS)
    logits4 = logits4.astype(np.float32)

    t_out = logits4[0:2].T + np.asarray(b1, np.float32)  # [256, 2]
    f_out = logits4[2:4].T + np.asarray(b2, np.float32)  # [256, 2]
    m = t_out.max(axis=1, keepdims=True)
    e = np.exp(t_out - m)
    conf = (e / e.sum(axis=1, keepdims=True)).max(axis=1)
    blended = 0.7 * t_out + 0.3 * f_out
    out = np.where((conf > THRESHOLD)[:, None], t_out, blended)
    return out.astype(np.float32)
